# revision 11
# baseline (speedup 1.0000x reference)
"""Trainium2 Bass kernel for nn_DMHA_47485158425056.

Fused dense-transformer block: Q/K projections, VQ-gated value path
(top-1 codebook lookup), causal attention, output projection.

Sharding over 8 NeuronCores: data-parallel on batch (2) x tensor-parallel on
head groups (4 heads of 16 per core).  Each core computes
  out_partial[b,g] = attn_heads_g(x[b]) @ Wo[:, cols_g].T
and the host sums the 4 partials per batch and adds bo.

All heavy matmuls run in float32r (fp32 rounded to 11-bit mantissa; full
PE-array rate).  The VQ argmax path (vq projection, similarities) runs in
exact fp32 so the top-1 index matches the fp32 reference.

Layouts (T = transposed so the d_model/contraction dim is on partitions):
  xtr  [D, S]    x[b].T, fp32r          (Q/K projection moving operand)
  xt   [D, S]    x[b].T, exact fp32     (vq projection moving operand)
  xn   [S, 512]  x[b][:, cols_g]        (value gate elementwise operand)
  wqt/wkt [D, 512]  Wq[cols_g,:].T fp32r (projection stationary operands)
  wvqt [D, 128]  Wvq.T exact fp32
  vkt  [128, 64] V_keys.T exact fp32
  vemb [64, 512] V_embed[:, cols_g] fp32r
  wot  [512, D]  Wo[:, cols_g].T fp32r
Scores are computed transposed (k on partitions, q free) so the probs feed
attn@V directly as the moving operand with V natural-layout stationary --
no probability transposes.  Softmax skips max-subtraction (|scores/sqrt(d)|
is O(6) here; exp is safe in fp32) and the denominator is a ones-column
matmul over the summed probability tiles.
"""

import math

import numpy as np

import concourse.bass as bass
import concourse.mybir as mybir
import concourse.tile as tile
from concourse import bacc
from concourse.bass_utils import run_bass_kernel_spmd
from concourse.masks import make_identity

# Problem dims (hardcoded per contract)
BSZ, SEQ, D = 2, 2048, 2048
NH, HD, NV = 16, 128, 64
P = 128
NCORES = 8
NHL = 4  # heads per core
HCOL = NHL * HD  # 512 head-columns per core
SC = 512  # token chunk
NC_ = SEQ // SC  # 4 chunks
ND = D // P  # 16 d-model tiles
NT = SEQ // P  # 16 token tiles
SCALE = 1.0 / math.sqrt(HD)
MASK_W = 896  # [P, 896] master causal mask, slice [384-d : 896-d]

F32 = mybir.dt.float32
F32R = mybir.dt.float32r
EXP = mybir.ActivationFunctionType.Exp
IDENT = mybir.ActivationFunctionType.Identity


def _round_fp32r(a: np.ndarray) -> np.ndarray:
    """Round fp32 to the fp32r grid (11-bit mantissa, low 12 bits zero),
    round-to-nearest-even, matching walrus' fp32_to_fp32r."""
    u = np.ascontiguousarray(a).view(np.uint32)
    low = u & np.uint32(0xFFF)
    half = np.uint32(0x800)
    base = u & np.uint32(0xFFFFF000)
    inc = (low > half) | ((low == half) & (((u >> np.uint32(12)) & np.uint32(1)) != 0))
    return (base + (inc.astype(np.uint32) << np.uint32(12))).view(np.float32)


def _build_program(reps: int = 1):
    nc = bacc.Bacc(
        trn_type="TRN2", target_bir_lowering=False, debug=False, num_devices=NCORES
    )
    dram = {
        "xtr": nc.dram_tensor("xtr", [D, SEQ], F32R, kind="ExternalInput").ap(),
        "xt": nc.dram_tensor("xt", [D, SEQ], F32, kind="ExternalInput").ap(),
        "xn": nc.dram_tensor("xn", [SEQ, HCOL], F32, kind="ExternalInput").ap(),
        "wqt": nc.dram_tensor("wqt", [D, HCOL], F32R, kind="ExternalInput").ap(),
        "wkt": nc.dram_tensor("wkt", [D, HCOL], F32R, kind="ExternalInput").ap(),
        "wvqt": nc.dram_tensor("wvqt", [D, HD], F32, kind="ExternalInput").ap(),
        "vkt": nc.dram_tensor("vkt", [HD, NV], F32, kind="ExternalInput").ap(),
        "vemb": nc.dram_tensor("vemb", [NV, HCOL], F32R, kind="ExternalInput").ap(),
        "wot": nc.dram_tensor("wot", [HCOL, D], F32R, kind="ExternalInput").ap(),
        "mask": nc.dram_tensor("mask", [P, MASK_W], F32, kind="ExternalInput").ap(),
        "bqg": nc.dram_tensor("bqg", [P, NHL], F32, kind="ExternalInput").ap(),
        "bkg": nc.dram_tensor("bkg", [P, NHL], F32, kind="ExternalInput").ap(),
        "bvq": nc.dram_tensor("bvq", [P, 1], F32, kind="ExternalInput").ap(),
    }
    outp = nc.dram_tensor("outp", [SEQ, D], F32, kind="ExternalOutput").ap()

    def emit_body(tc):
        with (
            tc.tile_pool(name="consts", bufs=1) as consts,
            tc.tile_pool(name="persist", bufs=1) as persist,
            tc.tile_pool(name="stream", bufs=3) as stream,
            tc.tile_pool(name="work", bufs=3) as work,
            tc.tile_pool(name="psum", bufs=8, space="PSUM") as psum,
        ):
            # ---- constants ----
            ident = consts.tile([P, P], F32)
            make_identity(nc, ident)
            mask_sb = consts.tile([P, MASK_W], F32)
            nc.sync.dma_start(out=mask_sb, in_=dram["mask"])
            ones_r = consts.tile([P, 1], F32R)
            nc.vector.memset(ones_r.bitcast(F32), 1.0)
            bqg_sb = consts.tile([P, NHL], F32)
            nc.sync.dma_start(out=bqg_sb, in_=dram["bqg"])
            bkg_sb = consts.tile([P, NHL], F32)
            nc.sync.dma_start(out=bkg_sb, in_=dram["bkg"])
            bvq_sb = consts.tile([P, 1], F32)
            nc.sync.dma_start(out=bvq_sb, in_=dram["bvq"])
            vkt_sb = consts.tile([HD, NV], F32)
            nc.sync.dma_start(out=vkt_sb, in_=dram["vkt"])
            vemb_sb = consts.tile([NV, HCOL], F32R)
            nc.sync.dma_start(out=vemb_sb, in_=dram["vemb"])

            # ---- persistent activations ----
            kt_all = persist.tile([P, NHL, SEQ], F32R)  # K.T per head
            vnat = [
                persist.tile([P, HCOL], F32R, name=f"vnat{tt}") for tt in range(NT)
            ]  # gated V, natural layout, one tile per token-tile
            yt_all = persist.tile([P, NHL, SEQ], F32R)  # attn output, transposed

            # ================= Phase V1: vq projection (exact fp32) ========
            with tc.tile_pool(name="v1", bufs=1) as v1pool:
                wvqt_sb = v1pool.tile([P, ND, HD], F32)
                nc.sync.dma_start(
                    out=wvqt_sb, in_=dram["wvqt"].rearrange("(n p) h -> p n h", p=P)
                )
                vqt_sb = v1pool.tile([HD, SEQ], F32)
                vq_ps = [psum.tile([P, SC], F32, tag="bank", name=f"vq_ps{c}") for c in range(NC_)]
                for dt in range(ND):
                    for c in range(NC_):
                        xt_t = stream.tile([P, SC], F32, tag="xt")
                        nc.gpsimd.dma_start(
                            out=xt_t,
                            in_=dram["xt"][
                                dt * P : (dt + 1) * P, c * SC : (c + 1) * SC
                            ],
                        )
                        nc.tensor.matmul(
                            vq_ps[c],
                            wvqt_sb[:, dt, :],
                            xt_t,
                            start=(dt == 0),
                            stop=(dt == ND - 1),
                        )
                for c in range(NC_):
                    nc.scalar.activation(
                        out=vqt_sb[:, c * SC : (c + 1) * SC],
                        in_=vq_ps[c],
                        func=IDENT,
                        bias=bvq_sb,
                    )

                # ============ Phase V2: top-1 select + gated V =============
                for tt in range(NT):
                    sim_ps = psum.tile([P, NV], F32, tag="bank")
                    nc.tensor.matmul(
                        sim_ps,
                        vqt_sb[:, tt * P : (tt + 1) * P],
                        vkt_sb,
                        start=True,
                        stop=True,
                    )
                    nrmax = work.tile([P, 1], F32, tag="nrmax")
                    nc.vector.tensor_reduce(
                        nrmax,
                        sim_ps,
                        axis=mybir.AxisListType.X,
                        op=mybir.AluOpType.max,
                        negate=True,
                    )
                    shifted = work.tile([P, NV], F32, tag="shifted")
                    nc.scalar.activation(
                        out=shifted, in_=sim_ps, func=IDENT, bias=nrmax
                    )
                    oh = work.tile([P, NV], F32, tag="oh")
                    nc.vector.tensor_scalar(
                        oh, shifted, 0.0, None, op0=mybir.AluOpType.is_equal
                    )
                    oht_ps = psum.tile([NV, P], F32, tag="bank")
                    nc.tensor.transpose(oht_ps, oh, ident)
                    oht = work.tile([NV, P], F32R, tag="oht")
                    nc.scalar.copy(oht, oht_ps)
                    gsel_ps = psum.tile([P, HCOL], F32, tag="bank")
                    nc.tensor.matmul(gsel_ps, oht, vemb_sb, start=True, stop=True)
                    xn_t = stream.tile([P, HCOL], F32, tag="xn")
                    nc.gpsimd.dma_start(
                        out=xn_t, in_=dram["xn"][tt * P : (tt + 1) * P, :]
                    )
                    nc.vector.tensor_mul(vnat[tt], gsel_ps, xn_t)

            # ====== Main loop: Q/K projection chunk + causal attention =====
            with tc.tile_pool(name="qk", bufs=1) as qkpool:
                wqt_sb = qkpool.tile([P, ND, HCOL], F32R)
                nc.sync.dma_start(
                    out=wqt_sb, in_=dram["wqt"].rearrange("(n p) m -> p n m", p=P)
                )
                for c in range(NC_):
                    qt_ps = [psum.tile([P, SC], F32, tag="bank", name=f"qt_ps{i}") for i in range(NHL)]
                    kt_ps = [psum.tile([P, SC], F32, tag="bank", name=f"kt_ps{i}") for i in range(NHL)]
                    for dt in range(ND):
                        xtr_t = stream.tile([P, SC], F32R, tag="xtr")
                        nc.sync.dma_start(
                            out=xtr_t,
                            in_=dram["xtr"][
                                dt * P : (dt + 1) * P, c * SC : (c + 1) * SC
                            ],
                        )
                        wkt_t = stream.tile([P, HCOL], F32R, tag="wkt")
                        nc.sync.dma_start(
                            out=wkt_t, in_=dram["wkt"][dt * P : (dt + 1) * P, :]
                        )
                        for pt in range(NHL):
                            nc.tensor.matmul(
                                qt_ps[pt],
                                wqt_sb[:, dt, pt * HD : (pt + 1) * HD],
                                xtr_t,
                                start=(dt == 0),
                                stop=(dt == ND - 1),
                            )
                            nc.tensor.matmul(
                                kt_ps[pt],
                                wkt_t[:, pt * HD : (pt + 1) * HD],
                                xtr_t,
                                start=(dt == 0),
                                stop=(dt == ND - 1),
                            )
                    qt_cur = work.tile([P, NHL, SC], F32R, tag="qtc", bufs=2)
                    for pt in range(NHL):
                        nc.scalar.activation(
                            out=qt_cur[:, pt, :],
                            in_=qt_ps[pt],
                            func=IDENT,
                            bias=bqg_sb[:, pt : pt + 1],
                        )
                        nc.scalar.activation(
                            out=kt_all[:, pt, c * SC : (c + 1) * SC],
                            in_=kt_ps[pt],
                            func=IDENT,
                            bias=bkg_sb[:, pt : pt + 1],
                        )

                    # causal attention for q-chunk c, all local heads
                    nkt = 4 * c + 4
                    for h in range(NHL):
                        yt_ps = psum.tile([P, SC], F32, tag="bank")
                        dsum_ps = psum.tile([1, SC], F32, tag="bank")
                        for kt in range(nkt):
                            # diagonal tiles with offset d in {128,256}: the
                            # q < d columns are fully masked -- skip them.
                            # (d=384 stays full width: N=128 fp32r runs at
                            # 4 cyc/row, same cost as full 512 at 1 cyc.)
                            dmo = (kt - 4 * c) * P if kt >= 4 * c else 0
                            d0 = dmo if dmo in (P, 2 * P) else 0
                            sc_ps = psum.tile([P, SC], F32, tag="bank")
                            nc.tensor.matmul(
                                sc_ps[:, d0:],
                                kt_all[:, h, kt * P : (kt + 1) * P],
                                qt_cur[:, h, d0:],
                                start=True,
                                stop=True,
                            )
                            if kt >= 4 * c:
                                nc.vector.tensor_add(
                                    sc_ps[:, d0:],
                                    sc_ps[:, d0:],
                                    mask_sb[:, 384 - dmo + d0 : 896 - dmo],
                                )
                            probs = work.tile([P, SC], F32R, tag="probs")
                            nc.scalar.activation(
                                out=probs[:, d0:], in_=sc_ps[:, d0:], func=EXP, scale=SCALE
                            )
                            nc.tensor.matmul(
                                yt_ps[:, d0:],
                                vnat[kt][:, h * HD : (h + 1) * HD],
                                probs[:, d0:],
                                start=(kt == 0),
                                stop=(kt == nkt - 1),
                            )
                            nc.tensor.matmul(
                                dsum_ps[:, d0:],
                                ones_r,
                                probs[:, d0:],
                                start=(kt == 0),
                                stop=(kt == nkt - 1),
                            )
                        recip = work.tile([1, SC], F32, tag="recip")
                        nc.vector.reciprocal(recip, dsum_ps)
                        recip_b = work.tile([P, SC], F32, tag="recipb")
                        nc.gpsimd.partition_broadcast(recip_b, recip)
                        nc.vector.tensor_mul(
                            yt_all[:, h, c * SC : (c + 1) * SC],
                            yt_ps,
                            recip_b,
                        )



            # ================= Phase O: output projection ==================
            with tc.tile_pool(name="oproj", bufs=1) as opool:
                wot_sb = opool.tile([P, NHL, D], F32R)
                nc.sync.dma_start(
                    out=wot_sb, in_=dram["wot"].rearrange("(n p) m -> p n m", p=P)
                )
                for tt in range(NT):
                    for oc in range(NC_):
                        op_ps = psum.tile([P, SC], F32, tag="bank")
                        for h in range(NHL):
                            nc.tensor.matmul(
                                op_ps,
                                yt_all[:, h, tt * P : (tt + 1) * P],
                                wot_sb[:, h, oc * SC : (oc + 1) * SC],
                                start=(h == 0),
                                stop=(h == NHL - 1),
                            )
                        out_sb = work.tile([P, SC], F32, tag="outsb")
                        nc.scalar.copy(out_sb, op_ps)
                        nc.sync.dma_start(
                            out=outp[tt * P : (tt + 1) * P, oc * SC : (oc + 1) * SC],
                            in_=out_sb,
                        )

    with tile.TileContext(nc) as tc:
        for _rep in range(reps):
            emit_body(tc)

    nc.finalize()
    return nc


_PROGRAM_CACHE = {}


def _get_program(reps: int = 1):
    if reps not in _PROGRAM_CACHE:
        _PROGRAM_CACHE[reps] = _build_program(reps)
    return _PROGRAM_CACHE[reps]


def make_in_maps(x, Wq, bq, Wk, bk, Wvq, bvq, V_keys, V_embed, Wo, bo):
    """Per-core input shards (host-side prep)."""
    x = np.asarray(x, dtype=np.float32)
    mask = np.where(
        np.arange(MASK_W)[None, :] >= np.arange(P)[:, None] + 384,
        np.float32(0.0),
        np.float32(-1e9),
    ).astype(np.float32)
    wvqt = np.ascontiguousarray(np.asarray(Wvq, np.float32).T)
    vkt = np.ascontiguousarray(np.asarray(V_keys, np.float32).T)
    bvq_t = np.asarray(bvq, np.float32).reshape(P, 1)
    wqT = np.asarray(Wq, np.float32).T  # [in, out]
    wkT = np.asarray(Wk, np.float32).T
    in_maps = []
    xts, xtrs = {}, {}
    for b in range(BSZ):
        xt = np.ascontiguousarray(x[b].T)
        xts[b] = xt
        xtrs[b] = _round_fp32r(xt)
    for core in range(NCORES):
        b, g = core // NHL, core % NHL
        cols = slice(g * HCOL, (g + 1) * HCOL)
        in_maps.append(
            {
                "xtr": xtrs[b],
                "xt": xts[b],
                "xn": np.ascontiguousarray(x[b][:, cols]),
                "wqt": _round_fp32r(np.ascontiguousarray(wqT[:, cols])),
                "wkt": _round_fp32r(np.ascontiguousarray(wkT[:, cols])),
                "wvqt": wvqt,
                "vkt": vkt,
                "vemb": _round_fp32r(
                    np.ascontiguousarray(np.asarray(V_embed, np.float32)[:, cols])
                ),
                "wot": _round_fp32r(
                    np.ascontiguousarray(np.asarray(Wo, np.float32)[:, cols].T)
                ),
                "mask": mask,
                "bqg": np.ascontiguousarray(
                    np.asarray(bq, np.float32)[cols].reshape(NHL, P).T
                ),
                "bkg": np.ascontiguousarray(
                    np.asarray(bk, np.float32)[cols].reshape(NHL, P).T
                ),
                "bvq": bvq_t,
            }
        )
    return in_maps


def assemble_output(results, bo):
    bo = np.asarray(bo, np.float32)
    out = np.empty((BSZ, SEQ, D), np.float32)
    for b in range(BSZ):
        acc = results[b * NHL]["outp"].astype(np.float32).copy()
        for g in range(1, NHL):
            acc += results[b * NHL + g]["outp"]
        out[b] = acc + bo[None, :]
    return out


def kernel(x, Wq, bq, Wk, bk, Wvq, bvq, V_keys, V_embed, Wo, bo):
    nc = _get_program()
    in_maps = make_in_maps(x, Wq, bq, Wk, bk, Wvq, bvq, V_keys, V_embed, Wo, bo)
    res = run_bass_kernel_spmd(nc, in_maps, core_ids=list(range(NCORES)), trace=False)
    return assemble_output(res.results, bo)


# revision 12
# speedup vs baseline: 2.0819x; 2.0819x over previous
"""Trainium2 Bass kernel for nn_DMHA_47485158425056.

Fused dense-transformer block: Q/K projections, VQ-gated value path
(top-1 codebook lookup), causal attention, output projection.

Sharding over 8 NeuronCores: data-parallel on batch (2) x tensor-parallel on
head groups (4 heads of 16 per core).  Each core computes
  out_partial[b,g] = attn_heads_g(x[b]) @ Wo[:, cols_g].T
and the host sums the 4 partials per batch and adds bo.

All heavy matmuls run in float32r (fp32 rounded to 11-bit mantissa; full
PE-array rate).  The VQ argmax path (vq projection, similarities) runs in
exact fp32 so the top-1 index matches the fp32 reference.

Layouts (T = transposed so the d_model/contraction dim is on partitions):
  xtr  [D, S]    x[b].T, fp32r          (Q/K projection moving operand)
  xt   [D, S]    x[b].T, exact fp32     (vq projection moving operand)
  xn   [S, 512]  x[b][:, cols_g]        (value gate elementwise operand)
  wqt/wkt [D, 512]  Wq[cols_g,:].T fp32r (projection stationary operands)
  wvqt [D, 128]  Wvq.T exact fp32
  vkt  [128, 64] V_keys.T exact fp32
  vemb [64, 512] V_embed[:, cols_g] fp32r
  wot  [512, D]  Wo[:, cols_g].T fp32r
Scores are computed transposed (k on partitions, q free) so the probs feed
attn@V directly as the moving operand with V natural-layout stationary --
no probability transposes.  Softmax skips max-subtraction (|scores/sqrt(d)|
is O(6) here; exp is safe in fp32) and the denominator is a ones-column
matmul over the summed probability tiles.
"""

import math

import numpy as np

import concourse.bass as bass
import concourse.mybir as mybir
import concourse.tile as tile
from concourse import bacc
from concourse.bass_utils import run_bass_kernel_spmd
from concourse.masks import make_identity

# Problem dims (hardcoded per contract)
BSZ, SEQ, D = 2, 2048, 2048
NH, HD, NV = 16, 128, 64
P = 128
NCORES = 8
NHL = 4  # heads per core
HCOL = NHL * HD  # 512 head-columns per core
SC = 512  # token chunk
NC_ = SEQ // SC  # 4 chunks
ND = D // P  # 16 d-model tiles
NT = SEQ // P  # 16 token tiles
SCALE = 1.0 / math.sqrt(HD)
MASK_W = 896  # [P, 896] master causal mask, slice [384-d : 896-d]

F32 = mybir.dt.float32
F32R = mybir.dt.float32r
EXP = mybir.ActivationFunctionType.Exp
IDENT = mybir.ActivationFunctionType.Identity


def _round_fp32r(a: np.ndarray) -> np.ndarray:
    """Round fp32 to the fp32r grid (11-bit mantissa, low 12 bits zero),
    round-to-nearest-even, matching walrus' fp32_to_fp32r."""
    u = np.ascontiguousarray(a).view(np.uint32)
    low = u & np.uint32(0xFFF)
    half = np.uint32(0x800)
    base = u & np.uint32(0xFFFFF000)
    inc = (low > half) | ((low == half) & (((u >> np.uint32(12)) & np.uint32(1)) != 0))
    return (base + (inc.astype(np.uint32) << np.uint32(12))).view(np.float32)


def _build_program(reps: int = 1):
    nc = bacc.Bacc(
        trn_type="TRN2", target_bir_lowering=False, debug=False, num_devices=NCORES
    )
    dram = {
        "xtr": nc.dram_tensor("xtr", [D, SEQ], F32R, kind="ExternalInput").ap(),
        "xt": nc.dram_tensor("xt", [D, SEQ], F32, kind="ExternalInput").ap(),
        "xn": nc.dram_tensor("xn", [SEQ, HCOL], F32, kind="ExternalInput").ap(),
        "wqt": nc.dram_tensor("wqt", [D, HCOL], F32R, kind="ExternalInput").ap(),
        "wkt": nc.dram_tensor("wkt", [D, HCOL], F32R, kind="ExternalInput").ap(),
        "wvqt": nc.dram_tensor("wvqt", [D, HD], F32, kind="ExternalInput").ap(),
        "vkt": nc.dram_tensor("vkt", [HD, NV], F32, kind="ExternalInput").ap(),
        "vemb": nc.dram_tensor("vemb", [NV, HCOL], F32R, kind="ExternalInput").ap(),
        "wot": nc.dram_tensor("wot", [HCOL, D], F32R, kind="ExternalInput").ap(),
        "mask": nc.dram_tensor("mask", [P, MASK_W], F32, kind="ExternalInput").ap(),
        "bqg": nc.dram_tensor("bqg", [P, NHL], F32, kind="ExternalInput").ap(),
        "bkg": nc.dram_tensor("bkg", [P, NHL], F32, kind="ExternalInput").ap(),
        "bvq": nc.dram_tensor("bvq", [P, 1], F32, kind="ExternalInput").ap(),
    }
    outp = nc.dram_tensor("outp", [SEQ, D], F32, kind="ExternalOutput").ap()

    def emit_body(tc):
        with (
            tc.tile_pool(name="consts", bufs=1) as consts,
            tc.tile_pool(name="persist", bufs=1) as persist,
            tc.tile_pool(name="stream", bufs=3) as stream,
            tc.tile_pool(name="work", bufs=3) as work,
            tc.tile_pool(name="psum", bufs=8, space="PSUM") as psum,
        ):
            # ---- constants ----
            ident = consts.tile([P, P], F32)
            make_identity(nc, ident)
            mask_sb = consts.tile([P, MASK_W], F32)
            nc.sync.dma_start(out=mask_sb, in_=dram["mask"])
            ones_r = consts.tile([P, 1], F32R)
            nc.vector.memset(ones_r.bitcast(F32), 1.0)
            bqg_sb = consts.tile([P, NHL], F32)
            nc.sync.dma_start(out=bqg_sb, in_=dram["bqg"])
            bkg_sb = consts.tile([P, NHL], F32)
            nc.sync.dma_start(out=bkg_sb, in_=dram["bkg"])
            bvq_sb = consts.tile([P, 1], F32)
            nc.sync.dma_start(out=bvq_sb, in_=dram["bvq"])
            vkt_sb = consts.tile([HD, NV], F32)
            nc.sync.dma_start(out=vkt_sb, in_=dram["vkt"])
            vemb_sb = consts.tile([NV, HCOL], F32R)
            nc.sync.dma_start(out=vemb_sb, in_=dram["vemb"])

            # ---- persistent activations ----
            kt_all = persist.tile([P, NHL, SEQ], F32R)  # K.T per head
            vnat = [
                persist.tile([P, HCOL], F32R, name=f"vnat{tt}") for tt in range(NT)
            ]  # gated V, natural layout, one tile per token-tile
            yt_all = persist.tile([P, NHL, SEQ], F32R)  # attn output, transposed

            # ================= Phase V1: vq projection (exact fp32) ========
            with tc.tile_pool(name="v1", bufs=1) as v1pool:
                wvqt_sb = v1pool.tile([P, ND, HD], F32)
                nc.sync.dma_start(
                    out=wvqt_sb, in_=dram["wvqt"].rearrange("(n p) h -> p n h", p=P)
                )
                vqt_sb = v1pool.tile([HD, SEQ], F32)
                vq_ps = [psum.tile([P, SC], F32, tag="bank", name=f"vq_ps{c}") for c in range(NC_)]
                for dt in range(ND):
                    for c in range(NC_):
                        xt_t = stream.tile([P, SC], F32, tag="xt")
                        nc.gpsimd.dma_start(
                            out=xt_t,
                            in_=dram["xt"][
                                dt * P : (dt + 1) * P, c * SC : (c + 1) * SC
                            ],
                        )
                        nc.tensor.matmul(
                            vq_ps[c],
                            wvqt_sb[:, dt, :],
                            xt_t,
                            start=(dt == 0),
                            stop=(dt == ND - 1),
                        )
                for c in range(NC_):
                    nc.scalar.activation(
                        out=vqt_sb[:, c * SC : (c + 1) * SC],
                        in_=vq_ps[c],
                        func=IDENT,
                        bias=bvq_sb,
                    )

                # ============ Phase V2: top-1 select + gated V =============
                for tt in range(NT):
                    sim_ps = psum.tile([P, NV], F32, tag="bank")
                    nc.tensor.matmul(
                        sim_ps,
                        vqt_sb[:, tt * P : (tt + 1) * P],
                        vkt_sb,
                        start=True,
                        stop=True,
                    )
                    nrmax = work.tile([P, 1], F32, tag="nrmax")
                    nc.vector.tensor_reduce(
                        nrmax,
                        sim_ps,
                        axis=mybir.AxisListType.X,
                        op=mybir.AluOpType.max,
                        negate=True,
                    )
                    shifted = work.tile([P, NV], F32, tag="shifted")
                    nc.scalar.activation(
                        out=shifted, in_=sim_ps, func=IDENT, bias=nrmax
                    )
                    oh = work.tile([P, NV], F32, tag="oh")
                    nc.vector.tensor_scalar(
                        oh, shifted, 0.0, None, op0=mybir.AluOpType.is_equal
                    )
                    oht_ps = psum.tile([NV, P], F32, tag="bank")
                    nc.tensor.transpose(oht_ps, oh, ident)
                    oht = work.tile([NV, P], F32R, tag="oht")
                    nc.scalar.copy(oht, oht_ps)
                    gsel_ps = psum.tile([P, HCOL], F32, tag="bank")
                    nc.tensor.matmul(gsel_ps, oht, vemb_sb, start=True, stop=True)
                    xn_t = stream.tile([P, HCOL], F32, tag="xn")
                    nc.gpsimd.dma_start(
                        out=xn_t, in_=dram["xn"][tt * P : (tt + 1) * P, :]
                    )
                    nc.vector.tensor_mul(vnat[tt], gsel_ps, xn_t)

            # ====== Main loop: Q/K projection chunk + causal attention =====
            with tc.tile_pool(name="qk", bufs=1) as qkpool:
                wqt_sb = qkpool.tile([P, ND, HCOL], F32R)
                nc.sync.dma_start(
                    out=wqt_sb, in_=dram["wqt"].rearrange("(n p) m -> p n m", p=P)
                )
                for c in range(NC_):
                    qt_ps = [psum.tile([P, SC], F32, tag="bank", name=f"qt_ps{i}") for i in range(NHL)]
                    kt_ps = [psum.tile([P, SC], F32, tag="bank", name=f"kt_ps{i}") for i in range(NHL)]
                    for dt in range(ND):
                        xtr_t = stream.tile([P, SC], F32R, tag="xtr")
                        nc.sync.dma_start(
                            out=xtr_t,
                            in_=dram["xtr"][
                                dt * P : (dt + 1) * P, c * SC : (c + 1) * SC
                            ],
                        )
                        wkt_t = stream.tile([P, HCOL], F32R, tag="wkt")
                        nc.sync.dma_start(
                            out=wkt_t, in_=dram["wkt"][dt * P : (dt + 1) * P, :]
                        )
                        for pt in range(NHL):
                            nc.tensor.matmul(
                                qt_ps[pt],
                                wqt_sb[:, dt, pt * HD : (pt + 1) * HD],
                                xtr_t,
                                start=(dt == 0),
                                stop=(dt == ND - 1),
                            )
                            nc.tensor.matmul(
                                kt_ps[pt],
                                wkt_t[:, pt * HD : (pt + 1) * HD],
                                xtr_t,
                                start=(dt == 0),
                                stop=(dt == ND - 1),
                            )
                    qt_cur = work.tile([P, NHL, SC], F32R, tag="qtc", bufs=2)
                    for pt in range(NHL):
                        nc.scalar.activation(
                            out=qt_cur[:, pt, :],
                            in_=qt_ps[pt],
                            func=IDENT,
                            bias=bqg_sb[:, pt : pt + 1],
                        )
                        nc.scalar.activation(
                            out=kt_all[:, pt, c * SC : (c + 1) * SC],
                            in_=kt_ps[pt],
                            func=IDENT,
                            bias=bkg_sb[:, pt : pt + 1],
                        )

                    # causal attention for q-chunk c, all local heads
                    nkt = 4 * c + 4
                    for h in range(NHL):
                        yt_ps = psum.tile([P, SC], F32, tag="bank")
                        dsum_ps = psum.tile([1, SC], F32, tag="bank")
                        for kt in range(nkt):
                            sc_ps = psum.tile([P, SC], F32, tag="bank")
                            nc.tensor.matmul(
                                sc_ps,
                                kt_all[:, h, kt * P : (kt + 1) * P],
                                qt_cur[:, h, :],
                                start=True,
                                stop=True,
                            )
                            if kt >= 4 * c:
                                dmo = (kt - 4 * c) * P
                                nc.vector.tensor_add(
                                    sc_ps, sc_ps, mask_sb[:, 384 - dmo : 896 - dmo]
                                )
                            probs = work.tile([P, SC], F32R, tag="probs")
                            nc.scalar.activation(
                                out=probs, in_=sc_ps, func=EXP, scale=SCALE
                            )
                            nc.tensor.matmul(
                                yt_ps,
                                vnat[kt][:, h * HD : (h + 1) * HD],
                                probs,
                                start=(kt == 0),
                                stop=(kt == nkt - 1),
                            )
                            nc.tensor.matmul(
                                dsum_ps,
                                ones_r,
                                probs,
                                start=(kt == 0),
                                stop=(kt == nkt - 1),
                            )
                        recip = work.tile([1, SC], F32, tag="recip")
                        nc.vector.reciprocal(recip, dsum_ps)
                        recip_b = work.tile([P, SC], F32, tag="recipb")
                        nc.gpsimd.partition_broadcast(recip_b, recip)
                        nc.vector.tensor_mul(
                            yt_all[:, h, c * SC : (c + 1) * SC],
                            yt_ps,
                            recip_b,
                        )



            # ================= Phase O: output projection ==================
            with tc.tile_pool(name="oproj", bufs=1) as opool:
                wot_sb = opool.tile([P, NHL, D], F32R)
                nc.sync.dma_start(
                    out=wot_sb, in_=dram["wot"].rearrange("(n p) m -> p n m", p=P)
                )
                for tt in range(NT):
                    for oc in range(NC_):
                        op_ps = psum.tile([P, SC], F32, tag="bank")
                        for h in range(NHL):
                            nc.tensor.matmul(
                                op_ps,
                                yt_all[:, h, tt * P : (tt + 1) * P],
                                wot_sb[:, h, oc * SC : (oc + 1) * SC],
                                start=(h == 0),
                                stop=(h == NHL - 1),
                            )
                        out_sb = work.tile([P, SC], F32, tag="outsb")
                        nc.scalar.copy(out_sb, op_ps)
                        nc.sync.dma_start(
                            out=outp[tt * P : (tt + 1) * P, oc * SC : (oc + 1) * SC],
                            in_=out_sb,
                        )

    with tile.TileContext(nc) as tc:
        for _rep in range(reps):
            emit_body(tc)

    nc.finalize()
    return nc


_PROGRAM_CACHE = {}


def _get_program(reps: int = 1):
    if reps not in _PROGRAM_CACHE:
        _PROGRAM_CACHE[reps] = _build_program(reps)
    return _PROGRAM_CACHE[reps]


def make_in_maps(x, Wq, bq, Wk, bk, Wvq, bvq, V_keys, V_embed, Wo, bo):
    """Per-core input shards (host-side prep)."""
    x = np.asarray(x, dtype=np.float32)
    mask = np.where(
        np.arange(MASK_W)[None, :] >= np.arange(P)[:, None] + 384,
        np.float32(0.0),
        np.float32(-1e9),
    ).astype(np.float32)
    wvqt = np.ascontiguousarray(np.asarray(Wvq, np.float32).T)
    vkt = np.ascontiguousarray(np.asarray(V_keys, np.float32).T)
    bvq_t = np.asarray(bvq, np.float32).reshape(P, 1)
    wqT = np.asarray(Wq, np.float32).T  # [in, out]
    wkT = np.asarray(Wk, np.float32).T
    in_maps = []
    xts, xtrs = {}, {}
    for b in range(BSZ):
        xt = np.ascontiguousarray(x[b].T)
        xts[b] = xt
        xtrs[b] = _round_fp32r(xt)
    for core in range(NCORES):
        b, g = core // NHL, core % NHL
        cols = slice(g * HCOL, (g + 1) * HCOL)
        in_maps.append(
            {
                "xtr": xtrs[b],
                "xt": xts[b],
                "xn": np.ascontiguousarray(x[b][:, cols]),
                "wqt": _round_fp32r(np.ascontiguousarray(wqT[:, cols])),
                "wkt": _round_fp32r(np.ascontiguousarray(wkT[:, cols])),
                "wvqt": wvqt,
                "vkt": vkt,
                "vemb": _round_fp32r(
                    np.ascontiguousarray(np.asarray(V_embed, np.float32)[:, cols])
                ),
                "wot": _round_fp32r(
                    np.ascontiguousarray(np.asarray(Wo, np.float32)[:, cols].T)
                ),
                "mask": mask,
                "bqg": np.ascontiguousarray(
                    np.asarray(bq, np.float32)[cols].reshape(NHL, P).T
                ),
                "bkg": np.ascontiguousarray(
                    np.asarray(bk, np.float32)[cols].reshape(NHL, P).T
                ),
                "bvq": bvq_t,
            }
        )
    return in_maps


def assemble_output(results, bo):
    bo = np.asarray(bo, np.float32)
    out = np.empty((BSZ, SEQ, D), np.float32)
    for b in range(BSZ):
        acc = results[b * NHL]["outp"].astype(np.float32).copy()
        for g in range(1, NHL):
            acc += results[b * NHL + g]["outp"]
        out[b] = acc + bo[None, :]
    return out


def kernel(x, Wq, bq, Wk, bk, Wvq, bvq, V_keys, V_embed, Wo, bo):
    nc = _get_program()
    in_maps = make_in_maps(x, Wq, bq, Wk, bk, Wvq, bvq, V_keys, V_embed, Wo, bo)
    res = run_bass_kernel_spmd(nc, in_maps, core_ids=list(range(NCORES)), trace=False)
    return assemble_output(res.results, bo)
